# revision 1
# baseline (speedup 1.0000x reference)
"""Causal GQA self-attention (b=2, t=2048, 16 q-heads / 4 kv-heads, d=128,
RoPE + RMS-norm on q/k) distributed over 8 NeuronCores.

Sharding: core c = 4*b + g handles batch b and kv-group g (4 q-heads, 1 kv
head). Each core computes its q/k/v projections, RoPE + RMS, causal
attention in transposed-score layout, and a row-parallel partial o_proj
(wo rows for its heads). Host sums the 4 partials per batch (all-reduce
equivalent) and transposes back.

All matmuls run as float32r (full PE rate at free-dim 512, ~1.5e-4 rms rel
error for K=2048 contractions). Softmax needs no max-subtraction: q is
RMS-normalized and k's RMS factor is applied inside the exp via the
per-partition ACT scale, so scores/sqrt(128) are bounded by ~±11.3 and the
causal -10000 mask is realized as an exact multiplicative 0/1 mask on the
diagonal tiles (fully-masked tiles are skipped).
"""
import sys

sys.path.insert(0, "/opt/trn_rl_repo")

import numpy as np
import concourse.bass as bass
import concourse.bass_isa as bass_isa
from concourse import bacc
import concourse.mybir as mybir
import concourse.tile as tile
from concourse.bass_utils import run_bass_kernel_spmd
from concourse.masks import make_identity
from contextlib import ExitStack

P = 128
B, T, E = 2, 2048, 2048
NH = 4            # q heads per core
D = 128           # head dim
DQ = NH * D       # per-core q width
DKV = 128         # per-core kv width
TT = 512          # t-tile (projection output / attention query tile)
NTT = T // TT     # 4
NJT = T // P      # 16 key tiles
NEC = E // P      # 16 contraction chunks
NQ = 8            # x chunks per t-tile in phase 1
EPS = 1e-6
ROPE_BASE = 100000.0
F32 = mybir.dt.float32
F32R = mybir.dt.float32r
AF = mybir.ActivationFunctionType

_cache = {}


def _build():
    nc = bacc.Bacc("TRN2", target_bir_lowering=False, debug=False)
    xT_d = nc.dram_tensor("xT", [E, T], F32R, kind="ExternalInput")
    wq_d = nc.dram_tensor("wq", [E, DQ], F32R, kind="ExternalInput")
    wk_d = nc.dram_tensor("wk", [E, DKV], F32R, kind="ExternalInput")
    wv_d = nc.dram_tensor("wv", [E, DKV], F32R, kind="ExternalInput")
    wo_d = nc.dram_tensor("wo", [DQ, E], F32R, kind="ExternalInput")
    cos_d = nc.dram_tensor("cosf", [P, T], F32, kind="ExternalInput")
    sin_d = nc.dram_tensor("sinf", [P, T], F32, kind="ExternalInput")
    msk_d = nc.dram_tensor("msk", [P, P], F32R, kind="ExternalInput")
    ones_d = nc.dram_tensor("ones", [P, 1], F32R, kind="ExternalInput")
    out_d = nc.dram_tensor("outT", [E, T], F32, kind="ExternalOutput")

    H = D // 2  # rope half

    with ExitStack() as ctx:
        tc = ctx.enter_context(tile.TileContext(nc))
        persist = ctx.enter_context(tc.tile_pool(name="persist", bufs=1))

        # ---- persistent tiles (both phases) ----
        msk_sb = persist.tile([P, P], F32R, tag="msk")
        qT_sb = persist.tile([P, NH, T], F32R, tag="qT")
        kT_sb = persist.tile([P, T], F32R, tag="kT")
        v_sb = persist.tile([P, NJT, D], F32R, tag="v")
        ones_sb = persist.tile([P, 1], F32R, tag="ones")
        nc.sync.dma_start(ones_sb, ones_d.ap())
        epsq_sb = persist.tile([P, 1], F32, tag="epsq")
        nc.vector.memset(epsq_sb, EPS)
        ident_sb = persist.tile([P, P], F32, tag="ident")
        make_identity(nc, ident_sb)

        # ================= phase 1: projections + rope + rms =================
        with tc.tile_pool(name="wpool", bufs=1) as wpool, \
             tc.tile_pool(name="xpool", bufs=6) as xpool, \
             tc.tile_pool(name="csp", bufs=2) as csp, \
             tc.tile_pool(name="work1", bufs=4) as work1, \
             tc.tile_pool(name="rows1", bufs=3) as rows1, \
             tc.tile_pool(name="ps_q", bufs=7, space="PSUM") as ps_q, \
             tc.tile_pool(name="ps_vt", bufs=1, space="PSUM") as ps_vt:

            wq_sb = wpool.tile([P, NEC, DQ], F32R, tag="wq")
            wk_sb = wpool.tile([P, NEC, DKV], F32R, tag="wk")
            wv_sb = wpool.tile([P, NEC, DKV], F32R, tag="wv")
            for w_sb, w_d in ((wq_sb, wq_d), (wk_sb, wk_d), (wv_sb, wv_d)):
                wr = w_d.ap().rearrange("(c p) m -> p c m", p=P)
                for cc in range(0, NEC, 4):
                    nc.sync.dma_start(w_sb[:, cc : cc + 4, :], wr[:, cc : cc + 4, :])

            for tt in range(NTT):
                ts_ = slice(tt * TT, (tt + 1) * TT)
                ctt = csp.tile([P, TT], F32, tag="ctt")
                stt = csp.tile([P, TT], F32, tag="stt")
                nc.sync.dma_start(ctt, cos_d.ap()[:, ts_])
                nc.sync.dma_start(stt, sin_d.ap()[:, ts_])

                # 6 co-accumulating chains: 4 q heads, k, v
                chains = [(wq_sb, h * D, (h + 1) * D) for h in range(NH)]
                chains.append((wk_sb, 0, DKV))
                chains.append((wv_sb, 0, DKV))
                pts = [ps_q.tile([P, TT], F32, tag="q", name=f"pq{i}")
                       for i in range(6)]
                EC4 = NEC // NQ
                for c4 in range(NQ):
                    xq = xpool.tile([P, EC4, TT], F32R, tag="xh")
                    nc.sync.dma_start(
                        xq,
                        xT_d.ap()[c4 * (E // NQ) : (c4 + 1) * (E // NQ), ts_]
                        .rearrange("(c p) t -> p c t", p=P))
                    for ci, (w_sb, lo, hi) in enumerate(chains):
                        for c in range(EC4):
                            ec = c4 * EC4 + c
                            nc.tensor.matmul(pts[ci], w_sb[:, ec, lo:hi],
                                             xq[:, c, :], start=(ec == 0),
                                             stop=(ec == NEC - 1))

                # q heads and k share the same rms+rope pipeline
                dsts = [qT_sb[:, h, ts_] for h in range(NH)] + [kT_sb[:, ts_]]
                for ci, dst in enumerate(dsts):
                    pq = pts[ci]
                    # ACT drains psum (copy + square) so the psum slot frees
                    # fast and all DVE rope ops run SBUF-only (2x mode)
                    pqs = work1.tile([P, TT], F32, tag="pqs")
                    nc.scalar.copy(pqs, pq)
                    qsq = work1.tile([P, TT], F32, tag="qsq")
                    nc.scalar.square(qsq, pq)
                    # rms factor from pre-rope values (rope preserves norms)
                    ssb = work1.tile([P, TT], F32, tag="ssb")
                    nc.gpsimd.partition_all_reduce(ssb, qsq, channels=P,
                                                   reduce_op=bass_isa.ReduceOp.add)
                    srow = rows1.tile([1, TT], F32, tag="srow")
                    nc.scalar.activation(srow, ssb[0:1, :], AF.Sqrt,
                                         bias=epsq_sb[0:1, :], scale=1.0 / D)
                    rrow = rows1.tile([1, TT], F32, tag="rrow")
                    nc.vector.reciprocal(rrow, srow)
                    rbc = work1.tile([P, TT], F32, tag="rbc")
                    nc.gpsimd.partition_broadcast(rbc, rrow)
                    rp = work1.tile([P, TT], F32, tag="rp")
                    rt = work1.tile([P, TT], F32, tag="rt")
                    nc.vector.tensor_mul(rp, pqs, ctt)
                    nc.vector.tensor_mul(rt[0:H], pqs[H:D], stt[H:D])
                    nc.vector.tensor_mul(rt[H:D], pqs[0:H], stt[0:H])
                    nc.vector.tensor_add(rp, rp, rt)
                    nc.vector.tensor_mul(dst, rp, rbc)

                # v: PE-transpose into [t, d] chunks
                pv = pts[5]
                vtmp = work1.tile([P, TT], F32, tag="vtmp")
                nc.scalar.copy(vtmp, pv)
                for j4 in range(TT // P):
                    pvt = ps_vt.tile([P, P], F32, tag="vt")
                    nc.tensor.transpose(pvt, vtmp[:, j4 * P : (j4 + 1) * P],
                                        ident_sb)
                    nc.vector.tensor_copy(v_sb[:, tt * 4 + j4, :], pvt)

        # ================= phase 2+3: attention + o_proj =================
        with tc.tile_pool(name="wop", bufs=1) as wop, \
             tc.tile_pool(name="ytp", bufs=2) as ytp, \
             tc.tile_pool(name="expp", bufs=6) as expp, \
             tc.tile_pool(name="ostg", bufs=3) as ostg, \
             tc.tile_pool(name="work2", bufs=3) as work2, \
             tc.tile_pool(name="rows2", bufs=2) as rows2, \
             tc.tile_pool(name="ps_s", bufs=4, space="PSUM") as ps_s, \
             tc.tile_pool(name="ps_y", bufs=2, space="PSUM") as ps_y, \
             tc.tile_pool(name="ps_d", bufs=1, space="PSUM") as ps_d, \
             tc.tile_pool(name="ps_o", bufs=1, space="PSUM") as ps_o:

            nc.sync.dma_start(msk_sb, msk_d.ap())
            wo_sb = wop.tile([P, DQ // P, E], F32R, tag="wo")
            nc.sync.dma_start(wo_sb, wo_d.ap().rearrange("(c p) e -> p c e", p=P))

            for it in range(NTT):
                its = slice(it * TT, (it + 1) * TT)
                yt = ytp.tile([P, NH, TT], F32R, tag="yt")
                for h in range(NH):
                    py = ps_y.tile([P, TT], F32, tag="y")
                    pd = ps_d.tile([1, TT], F32, tag="d")
                    jlast = 4 * it + 3
                    for jt in range(jlast + 1):
                        pss_ = ps_s.tile([P, TT], F32, tag="s")
                        nc.tensor.matmul(pss_, kT_sb[:, jt * P : (jt + 1) * P],
                                         qT_sb[:, h, its], start=True, stop=True)
                        ex = expp.tile([P, TT], F32R, tag="ex")
                        nc.scalar.activation(ex, pss_, AF.Exp,
                                             scale=float(1.0 / np.sqrt(D)))
                        if jt >= 4 * it:
                            # diagonal tile: cols < 128*o are fully masked,
                            # cols >= 128*(o+1) fully valid; only the 128-wide
                            # window straddles the diagonal
                            o_ = jt - 4 * it
                            if o_ > 0:
                                nc.vector.tensor_scalar_mul(
                                    ex[:, 0 : P * o_], ex[:, 0 : P * o_], 0.0)
                            nc.vector.tensor_mul(
                                ex[:, P * o_ : P * (o_ + 1)],
                                ex[:, P * o_ : P * (o_ + 1)],
                                msk_sb[:, :])
                        nc.tensor.matmul(py, v_sb[:, jt, :], ex,
                                         start=(jt == 0), stop=(jt == jlast))
                        nc.tensor.matmul(pd, ones_sb, ex,
                                         start=(jt == 0), stop=(jt == jlast))
                    rd = rows2.tile([1, TT], F32, tag="rd")
                    nc.vector.reciprocal(rd, pd)
                    rdb = work2.tile([P, TT], F32, tag="rdb")
                    nc.gpsimd.partition_broadcast(rdb, rd)
                    nc.vector.tensor_mul(yt[:, h, :], py, rdb)
                for e in range(NEC):
                    po = ps_o.tile([P, TT], F32, tag="o")
                    for c in range(DQ // P):
                        nc.tensor.matmul(po, wo_sb[:, c, e * P : (e + 1) * P],
                                         yt[:, c, :], start=(c == 0),
                                         stop=(c == DQ // P - 1))
                    og = ostg.tile([P, TT], F32, tag="og")
                    nc.vector.tensor_copy(og, po)
                    nc.sync.dma_start(out_d.ap()[e * P : (e + 1) * P, its], og)

    nc.compile()
    return nc


def _tables():
    half = D // 2
    inv_freq = 1.0 / (ROPE_BASE ** (np.arange(half, dtype=np.float64) / half))
    freqs = np.arange(T, dtype=np.float64)[:, None] * inv_freq[None, :]  # [T, half]
    cosT = np.cos(freqs).T.astype(np.float32)  # [half, T]
    sinT = np.sin(freqs).T.astype(np.float32)
    cos_full = np.concatenate([cosT, cosT], axis=0)          # [P, T]
    # rows 0:64 hold -sinT (multiplies x1 into out[64:128]), rows 64:128
    # hold +sinT (multiplies x2 into out[0:64]) so DVE input base
    # partitions match the swapped-half reads.
    sin_signed = np.concatenate([-sinT, sinT], axis=0)       # [P, T]
    # diagonal-window mask: mask[p, i] = 1 if i >= p (one [P,P] window
    # suffices — fully-masked columns are zero-filled separately)
    msk = (np.arange(P)[None, :] >= np.arange(P)[:, None]).astype(np.float32)
    return cos_full, sin_signed, msk


def kernel(x, wq, wk, wv, wo):
    if "nc" not in _cache:
        _cache["nc"] = _build()
    nc = _cache["nc"]
    cos_full, sin_signed, msk = _tables()
    xTs = [np.ascontiguousarray(np.asarray(x)[b].T).astype(np.float32)
           for b in range(B)]
    wq, wk, wv, wo = (np.asarray(a, dtype=np.float32) for a in (wq, wk, wv, wo))
    in_maps = []
    for c in range(8):
        b, g = divmod(c, 4)
        in_maps.append({
            "xT": xTs[b],
            "wq": np.ascontiguousarray(wq[:, g * DQ : (g + 1) * DQ]),
            "wk": np.ascontiguousarray(wk[:, g * DKV : (g + 1) * DKV]),
            "wv": np.ascontiguousarray(wv[:, g * DKV : (g + 1) * DKV]),
            "wo": np.ascontiguousarray(wo[g * DQ : (g + 1) * DQ, :]),
            "cosf": cos_full,
            "sinf": sin_signed,
            "msk": msk,
            "ones": np.ones((P, 1), dtype=np.float32),
        })
    res = run_bass_kernel_spmd(nc, in_maps, core_ids=list(range(8)))
    out = np.zeros((B, T, E), dtype=np.float32)
    for c in range(8):
        b = c // 4
        out[b] += res.results[c]["outT"].T
    return out



# revision 17
# speedup vs baseline: 1.2211x; 1.2211x over previous
"""Causal GQA self-attention (b=2, t=2048, 16 q-heads / 4 kv-heads, d=128,
RoPE + RMS-norm on q/k) distributed over 8 NeuronCores.

Sharding: core c = 4*b + g handles batch b and kv-group g (4 q-heads, 1 kv
head). Each core computes its q/k/v projections, RoPE + RMS, causal
attention in transposed-score layout, and a row-parallel partial o_proj
(wo rows for its heads). Host sums the 4 partials per batch (all-reduce
equivalent) and transposes back.

v5 design notes:
- all matmul operands bf16 (1 cycle/row PE rate at any free width, half
  DMA bytes); PSUM accumulation stays f32.
- q/k/v SBUF tensors are per-t-tile so attention reads only depend on that
  tile's rope writes.
- causal mask applied on the PE: an accumulating matmul adds
  -10000 * strict_lower (ident^T @ mneg) onto the diagonal score window in
  PSUM, so exp sees masked scores and DVE is off the attention critical
  path.
- softmax denominator via free-size-1 column matmuls into a [128,4] PSUM
  accumulator; per-head normalization deferred one head so its latency
  hides under the next head's matmuls.
- diagonal score tiles trimmed to valid query columns; jt loop runs
  diagonal-first so PSUM start/stop stays consistent.
- t-tile 3's rope/rms pipeline (all SBUF-side work) is deferred into phase
  2 after attention(0) — only its PSUM drains run at the phase boundary, so
  attention's exps aren't queued behind the rope tail on ACT. Drains are
  split ACT/DVE to halve the boundary queue.
- o_proj(it-1) e-tiles are interleaved into attention(it)'s head loop so
  ACT-bound attention stretches keep the PE fed.
- weight DMAs split per chunk group and interleaved with the first x tiles.
"""
import sys

sys.path.insert(0, "/opt/trn_rl_repo")

import numpy as np
import concourse.bass as bass
import concourse.bass_isa as bass_isa
from concourse import bacc
import concourse.mybir as mybir
import concourse.tile as tile
from concourse.bass_utils import run_bass_kernel_spmd
from concourse.masks import make_identity
from contextlib import ExitStack

P = 128
B, T, E = 2, 2048, 2048
NH = 4            # q heads per core
D = 128           # head dim
DQ = NH * D       # per-core q width
DKV = 128         # per-core kv width
TT = 512          # t-tile (projection output / attention query tile)
NTT = T // TT     # 4
NJT = T // P      # 16 key tiles
NEC = E // P      # 16 contraction chunks
NQ = 8            # x chunks per t-tile in phase 1
EC4 = NEC // NQ   # 2
EPS = 1e-6
ROPE_BASE = 100000.0
MASKV = -10000.0
F32 = mybir.dt.float32
BF = mybir.dt.bfloat16
AF = mybir.ActivationFunctionType

_cache = {}


def _build():
    nc = bacc.Bacc("TRN2", target_bir_lowering=False, debug=False)
    # all inputs host-rearranged to partition-contiguous layouts so every
    # DMA is one descriptor per partition
    xT_d = nc.dram_tensor("xT", [P, NTT, NEC, TT], BF, kind="ExternalInput")
    wq_d = nc.dram_tensor("wq", [P, NEC, DQ], BF, kind="ExternalInput")
    wk_d = nc.dram_tensor("wk", [P, NEC, DKV], BF, kind="ExternalInput")
    wv_d = nc.dram_tensor("wv", [P, NEC, DKV], BF, kind="ExternalInput")
    wo_d = nc.dram_tensor("wo", [P, DQ // P, E], BF, kind="ExternalInput")
    cos_d = nc.dram_tensor("cosf", [P, T], F32, kind="ExternalInput")
    sin_d = nc.dram_tensor("sinf", [P, T], F32, kind="ExternalInput")
    msk_d = nc.dram_tensor("msk", [P, P], BF, kind="ExternalInput")
    ones_d = nc.dram_tensor("ones", [P, 1], BF, kind="ExternalInput")
    out_d = nc.dram_tensor("outT", [E, T], BF, kind="ExternalOutput")

    H = D // 2  # rope half

    with ExitStack() as ctx:
        tc = ctx.enter_context(tile.TileContext(nc))
        persist = ctx.enter_context(tc.tile_pool(name="persist", bufs=1))

        # ---- persistent tiles (both phases) ----
        # strict-lower -10000 additive mask for the diagonal window
        msk_sb = persist.tile([P, P], BF, tag="msk")
        ones_sb = persist.tile([P, 1], BF, tag="ones")
        nc.sync.dma_start(ones_sb, ones_d.ap())
        # per-t-tile q/k/v so attention deps don't serialize on later tiles
        qT_t = [persist.tile([P, NH, TT], BF, tag=f"qT{t}", name=f"qT{t}")
                for t in range(NTT)]
        kT_t = [persist.tile([P, TT], BF, tag=f"kT{t}", name=f"kT{t}")
                for t in range(NTT)]
        v_t = [persist.tile([P, 4, D], BF, tag=f"v{t}", name=f"v{t}")
               for t in range(NTT)]
        epsq_sb = persist.tile([P, 1], F32, tag="epsq")
        nc.vector.memset(epsq_sb, EPS)
        identb_sb = persist.tile([P, P], BF, tag="identb")
        make_identity(nc, identb_sb)
        identf_sb = persist.tile([P, P], F32, tag="identf")
        make_identity(nc, identf_sb)
        # bridge tiles: t-tile 3's projections + rope tables survive the
        # phase-1 pool scope; its rope pipeline runs inside phase 2
        pqs3 = [persist.tile([P, TT], F32, tag=f"pqs3_{i}", name=f"pqs3_{i}")
                for i in range(5)]
        ct3 = persist.tile([P, TT], F32, tag="ct3")
        st3 = persist.tile([P, TT], F32, tag="st3")
        vt3 = persist.tile([P, TT], BF, tag="vt3")

        def rope_rows(pqs, qsq, dst, ctt, stt, scr, rows):
            """rms factor + rope from drained projections (pqs) -> dst.
            partition_all_reduce leaves the sum in every partition, so the
            rms scale is computed full-tile (ACT sqrt + DVE recip) with no
            row extraction / broadcast stage."""
            ssb = scr.tile([P, TT], F32, tag="ssb")
            nc.gpsimd.partition_all_reduce(ssb, qsq, channels=P,
                                           reduce_op=bass_isa.ReduceOp.add)
            srt = scr.tile([P, TT], F32, tag="srt")
            nc.scalar.activation(srt, ssb, AF.Sqrt,
                                 bias=epsq_sb, scale=1.0 / D)
            rbc = scr.tile([P, TT], F32, tag="rbc")
            nc.vector.reciprocal(rbc, srt)
            rp = scr.tile([P, TT], F32, tag="rp")
            rt = scr.tile([P, TT], F32, tag="rt")
            nc.vector.tensor_mul(rp, pqs, ctt)
            nc.vector.tensor_mul(rt[0:H], pqs[H:D], stt[H:D])
            nc.vector.tensor_mul(rt[H:D], pqs[0:H], stt[0:H])
            nc.vector.tensor_add(rp, rp, rt)
            nc.vector.tensor_mul(dst, rp, rbc)  # f32 -> bf16 cast

        # ================= phase 1: projections (+ rope for tt 0-2) =========
        with tc.tile_pool(name="wpool", bufs=1) as wpool, \
             tc.tile_pool(name="xpool", bufs=6) as xpool, \
             tc.tile_pool(name="csp", bufs=2) as csp, \
             tc.tile_pool(name="work1", bufs=4) as work1, \
             tc.tile_pool(name="rows1", bufs=3) as rows1, \
             tc.tile_pool(name="ps_q", bufs=7, space="PSUM") as ps_q, \
             tc.tile_pool(name="ps_vt", bufs=1, space="PSUM") as ps_vt:

            wq_sb = wpool.tile([P, NEC, DQ], BF, tag="wq")
            wk_sb = wpool.tile([P, NEC, DKV], BF, tag="wk")
            wv_sb = wpool.tile([P, NEC, DKV], BF, tag="wv")
            wrs = [(w_sb, w_d.ap())
                   for w_sb, w_d in ((wq_sb, wq_d), (wk_sb, wk_d), (wv_sb, wv_d))]

            def wgroup(lo, n):  # stage weight chunks lo..lo+n
                for w_sb, wr in wrs:
                    nc.sync.dma_start(w_sb[:, lo : lo + n, :],
                                      wr[:, lo : lo + n, :])

            for tt in range(NTT):
                ts_ = slice(tt * TT, (tt + 1) * TT)
                if tt == 3:
                    ctt, stt = ct3, st3
                else:
                    ctt = csp.tile([P, TT], F32, tag="ctt")
                    stt = csp.tile([P, TT], F32, tag="stt")
                if tt != 0:
                    nc.sync.dma_start(ctt, cos_d.ap()[:, ts_])
                    nc.sync.dma_start(stt, sin_d.ap()[:, ts_])
                else:
                    # first-need DMAs lead the queue: chunks 0-1 + first x
                    # tile gate the first matmul; everything else follows
                    wgroup(0, 2)

                # 6 co-accumulating chains: 4 q heads, k, v
                chains = [(wq_sb, h * D, (h + 1) * D) for h in range(NH)]
                chains.append((wk_sb, 0, DKV))
                chains.append((wv_sb, 0, DKV))
                pts = [ps_q.tile([P, TT], F32, tag="q", name=f"pq{i}")
                       for i in range(6)]
                for c4 in range(NQ):
                    xq = xpool.tile([P, EC4, TT], BF, tag="xh")
                    nc.sync.dma_start(
                        xq, xT_d.ap()[:, tt, c4 * EC4 : (c4 + 1) * EC4, :])
                    # stream remaining weight groups behind the early x tiles
                    if tt == 0:
                        if c4 == 0:
                            wgroup(2, 2)
                            nc.sync.dma_start(ctt, cos_d.ap()[:, ts_])
                            nc.sync.dma_start(stt, sin_d.ap()[:, ts_])
                        elif c4 in (1, 3, 5):
                            wgroup(4 * (c4 + 1) // 2, 4)
                    for ci, (w_sb, lo, hi) in enumerate(chains):
                        for c in range(EC4):
                            ec = c4 * EC4 + c
                            nc.tensor.matmul(pts[ci], w_sb[:, ec, lo:hi],
                                             xq[:, c, :], start=(ec == 0),
                                             stop=(ec == NEC - 1))

                # v: drain its psum first (single ACT copy, frees the bank)
                pv = pts[5]
                vtmp = vt3 if tt == 3 else work1.tile([P, TT], BF, tag="vtmp")
                nc.scalar.copy(vtmp, pv)

                if tt == 3:
                    # boundary: only drain the psums (split ACT/DVE so the
                    # ACT FIFO stays short); rope runs inside phase 2
                    for i, ci in enumerate((3, 4, 0, 1, 2)):
                        if i < 2:
                            nc.scalar.copy(pqs3[ci], pts[ci])
                        else:
                            nc.vector.tensor_copy(pqs3[ci], pts[ci])
                    continue

                # q heads and k share the same rms+rope pipeline
                dsts = [qT_t[tt][:, h, :] for h in range(NH)] + [kT_t[tt]]
                for ci in range(5):
                    pq = pts[ci]
                    # ACT drains psum (copy + square) so the psum slot frees
                    # fast and all DVE rope ops run SBUF-only (2x mode)
                    pqs = work1.tile([P, TT], F32, tag="pqs")
                    nc.scalar.copy(pqs, pq)
                    qsq = work1.tile([P, TT], F32, tag="qsq")
                    nc.scalar.square(qsq, pq)
                    rope_rows(pqs, qsq, dsts[ci], ctt, stt, work1, rows1)

                # v: PE-transpose into [t, d] chunks (bf16: 1 cycle/row)
                for j4 in range(TT // P):
                    pvt = ps_vt.tile([P, P], BF, tag="vt")
                    nc.tensor.transpose(pvt, vtmp[:, j4 * P : (j4 + 1) * P],
                                        identb_sb)
                    nc.vector.tensor_copy(v_t[tt][:, j4, :], pvt)

        # ================= phase 2+3: attention + o_proj =================
        with tc.tile_pool(name="wop", bufs=1) as wop, \
             tc.tile_pool(name="ytp", bufs=2) as ytp, \
             tc.tile_pool(name="expp", bufs=6) as expp, \
             tc.tile_pool(name="ostg", bufs=4) as ostg, \
             tc.tile_pool(name="work2", bufs=3) as work2, \
             tc.tile_pool(name="rows2", bufs=3) as rows2, \
             tc.tile_pool(name="ps_s", bufs=2, space="PSUM") as ps_s, \
             tc.tile_pool(name="ps_y", bufs=2, space="PSUM") as ps_y, \
             tc.tile_pool(name="ps_d", bufs=1, space="PSUM") as ps_d, \
             tc.tile_pool(name="ps_v3", bufs=1, space="PSUM") as ps_v3, \
             tc.tile_pool(name="ps_o", bufs=2, space="PSUM") as ps_o:

            nc.sync.dma_start(msk_sb, msk_d.ap())
            wo_sb = wop.tile([P, DQ // P, E], BF, tag="wo")
            nc.sync.dma_start(wo_sb, wo_d.ap())

            def finalize(h, yt, py, pd4):
                # 1/d back to a broadcast row: per-column PE transposes into
                # partition-0 rows (PSUM reads must be 32-aligned), then one
                # ACT copy to SBUF, gpsimd broadcast, and the normalization
                rq = work2.tile([P, 4], F32, tag="rq")
                nc.vector.reciprocal(rq, pd4)
                prt = ps_s.tile([P, TT], F32, tag="s", name="prt")
                for c in range(4):
                    # single start per bank: start=True lazily re-zeroes the
                    # whole 2KB zero-region, clobbering sibling columns
                    nc.tensor.matmul(prt[0:1, c * P : (c + 1) * P],
                                     rq[:, c : c + 1], identf_sb,
                                     is_transpose=True, start=(c == 0),
                                     stop=(c == 3), skip_group_check=True)
                rdrow = rows2.tile([1, TT], F32, tag="rdrow")
                nc.vector.tensor_copy(rdrow, prt[0:1, 0:TT])
                rdb = work2.tile([P, TT], F32, tag="rdb")
                nc.gpsimd.partition_broadcast(rdb, rdrow)
                nc.vector.tensor_mul(yt[:, h, :], py, rdb)  # f32 -> bf16

            def oproj_tiles(it, yt, es):
                its = slice(it * TT, (it + 1) * TT)
                for e in es:
                    po = ps_o.tile([P, TT], F32, tag="o")
                    for c in range(DQ // P):
                        nc.tensor.matmul(po, wo_sb[:, c, e * P : (e + 1) * P],
                                         yt[:, c, :], start=(c == 0),
                                         stop=(c == DQ // P - 1))
                    og = ostg.tile([P, TT], BF, tag="og")
                    # DVE drains: ACT is the scarce engine in phase 2
                    nc.vector.tensor_copy(og, po)
                    nc.sync.dma_start(out_d.ap()[e * P : (e + 1) * P, its], og)

            def attn(it, prev_yt):
                """attention for tile it; o_proj(it-1) e-tiles interleaved
                after each head so ACT-bound stretches keep the PE fed."""
                yt = ytp.tile([P, NH, TT], BF, tag="yt")
                pend = None
                for h in range(NH):
                    py = ps_y.tile([P, TT], F32, tag="y")
                    pd4 = ps_d.tile([P, 4], F32, tag="d")
                    # diagonal tiles first (ascending o), trimmed to the
                    # valid query columns, then the full non-diag tiles
                    seq = [(4 * it + o, TT - P * o, P * o) for o in range(4)]
                    seq += [(jt, TT, 0) for jt in range(4 * it)]
                    n = len(seq)
                    for si, (jt, w, lo) in enumerate(seq):
                        first, last = si == 0, si == n - 1
                        diag = jt >= 4 * it
                        o_ = jt - 4 * it if diag else 0
                        trimmed = diag
                        pss_ = ps_s.tile([P, TT], F32, tag="s")
                        nc.tensor.matmul(pss_[:, :w],
                                         kT_t[jt // 4][:, (jt % 4) * P : (jt % 4 + 1) * P],
                                         qT_t[it][:, h, lo:TT],
                                         start=True, stop=not diag,
                                         skip_group_check=True)
                        if diag:
                            # accumulate the causal mask on the PE: strict
                            # lower -10000 on the leading 128-col window
                            nc.tensor.matmul(pss_[:, 0:P], identb_sb,
                                             msk_sb[:, 0:P],
                                             start=False, stop=True,
                                             skip_group_check=True)
                        ex = expp.tile([P, TT], BF, tag="ex")
                        nc.scalar.activation(ex[:, :w], pss_[:, :w], AF.Exp,
                                             scale=float(1.0 / np.sqrt(D)))
                        nc.tensor.matmul(py[:, lo:TT], v_t[jt // 4][:, jt % 4, :],
                                         ex[:, :w], start=first, stop=last,
                                         skip_group_check=True)
                        # denominator: free-size-1 column matmuls per q chunk
                        for c in range(4):
                            if trimmed and c < o_:
                                continue
                            loc = P * (c - o_) if trimmed else P * c
                            # single start per bank (see finalize note)
                            nc.tensor.matmul(pd4[:, c : c + 1],
                                             ex[:, loc : loc + P], ones_sb,
                                             start=first and c == 0,
                                             stop=last,
                                             skip_group_check=True)
                    if pend is not None:
                        finalize(*pend)
                    if prev_yt is not None:
                        oproj_tiles(it - 1, prev_yt, range(4 * h, 4 * h + 4))
                    pend = (h, yt, py, pd4)
                finalize(*pend)
                return yt

            def late_rope_tt3():
                dsts = [qT_t[3][:, h, :] for h in range(NH)] + [kT_t[3]]
                for ci in range(5):
                    qsq = work2.tile([P, TT], F32, tag="qsq3")
                    nc.vector.tensor_mul(qsq, pqs3[ci], pqs3[ci])
                    rope_rows(pqs3[ci], qsq, dsts[ci], ct3, st3, work2, rows2)
                for j4 in range(TT // P):
                    pvt = ps_v3.tile([P, P], BF, tag="vt")
                    nc.tensor.transpose(pvt, vt3[:, j4 * P : (j4 + 1) * P],
                                        identb_sb)
                    nc.vector.tensor_copy(v_t[3][:, j4, :], pvt)

            yt0 = attn(0, None)
            late_rope_tt3()
            yt1 = attn(1, yt0)
            yt2 = attn(2, yt1)
            yt3 = attn(3, yt2)
            oproj_tiles(3, yt3, range(NEC))

    nc.compile()
    return nc


def _tables():
    half = D // 2
    inv_freq = 1.0 / (ROPE_BASE ** (np.arange(half, dtype=np.float64) / half))
    freqs = np.arange(T, dtype=np.float64)[:, None] * inv_freq[None, :]  # [T, half]
    cosT = np.cos(freqs).T.astype(np.float32)  # [half, T]
    sinT = np.sin(freqs).T.astype(np.float32)
    cos_full = np.concatenate([cosT, cosT], axis=0)          # [P, T]
    # rows 0:64 hold -sinT (multiplies x1 into out[64:128]), rows 64:128
    # hold +sinT (multiplies x2 into out[0:64]) so DVE input base
    # partitions match the swapped-half reads.
    sin_signed = np.concatenate([-sinT, sinT], axis=0)       # [P, T]
    # additive strict-lower MASKV mask for the diagonal window
    msk = np.where(np.arange(P)[None, :] < np.arange(P)[:, None],
                   MASKV, 0.0).astype(np.float32)
    return cos_full, sin_signed, msk


def kernel(x, wq, wk, wv, wo):
    import ml_dtypes
    bf = ml_dtypes.bfloat16
    if "nc" not in _cache:
        _cache["nc"] = _build()
    nc = _cache["nc"]
    cos_full, sin_signed, msk = _tables()
    def colshard(w, g, width):  # [E, M] -> [P, NEC, width] chunk layout
        ws = w[:, g * width : (g + 1) * width].astype(bf)
        return np.ascontiguousarray(ws.reshape(NEC, P, width).transpose(1, 0, 2))

    xTs = []
    for b in range(B):
        xT = np.asarray(x)[b].T.astype(bf)  # [E, T]
        xTs.append(np.ascontiguousarray(
            xT.reshape(NEC, P, NTT, TT).transpose(1, 2, 0, 3)))
    wq, wk, wv, wo = (np.asarray(a, dtype=np.float32) for a in (wq, wk, wv, wo))
    in_maps = []
    for c in range(8):
        b, g = divmod(c, 4)
        wos = wo[g * DQ : (g + 1) * DQ, :].astype(bf)
        in_maps.append({
            "xT": xTs[b],
            "wq": colshard(wq, g, DQ),
            "wk": colshard(wk, g, DKV),
            "wv": colshard(wv, g, DKV),
            "wo": np.ascontiguousarray(
                wos.reshape(DQ // P, P, E).transpose(1, 0, 2)),
            "cosf": cos_full,
            "sinf": sin_signed,
            "msk": msk.astype(bf),
            "ones": np.ones((P, 1), dtype=bf),
        })
    res = run_bass_kernel_spmd(nc, in_maps, core_ids=list(range(8)))
    out = np.zeros((B, T, E), dtype=np.float32)
    for c in range(8):
        b = c // 4
        out[b] += res.results[c]["outT"].astype(np.float32).T
    return out


# revision 25
# speedup vs baseline: 1.2862x; 1.0532x over previous
"""Causal GQA self-attention (b=2, t=2048, 16 q-heads / 4 kv-heads, d=128,
RoPE + RMS-norm on q/k) distributed over 8 NeuronCores.

Sharding: core c = 4*b + g handles batch b and kv-group g (4 q-heads, 1 kv
head). Each core computes its q/k/v projections, RoPE + RMS, causal
attention in transposed-score layout, and a row-parallel partial o_proj
(wo rows for its heads). Host sums the 4 partials per batch (all-reduce
equivalent) and transposes back.

v5 design notes:
- all matmul operands bf16 (1 cycle/row PE rate at any free width, half
  DMA bytes); PSUM accumulation stays f32.
- q/k/v SBUF tensors are per-t-tile so attention reads only depend on that
  tile's rope writes.
- causal mask applied on the PE: an accumulating matmul adds
  -10000 * strict_lower (ident^T @ mneg) onto the diagonal score window in
  PSUM, so exp sees masked scores and DVE is off the attention critical
  path.
- softmax denominator via free-size-1 column matmuls into a [128,4] PSUM
  accumulator; per-head normalization deferred one head so its latency
  hides under the next head's matmuls.
- diagonal score tiles trimmed to valid query columns; jt loop runs
  diagonal-first so PSUM start/stop stays consistent.
- t-tile 3's rope/rms pipeline (all SBUF-side work) is deferred into phase
  2 after attention(0) — only its PSUM drains run at the phase boundary, so
  attention's exps aren't queued behind the rope tail on ACT. Drains are
  split ACT/DVE to halve the boundary queue.
- o_proj(it-1) e-tiles are interleaved into attention(it)'s head loop so
  ACT-bound attention stretches keep the PE fed.
- weight DMAs split per chunk group and interleaved with the first x tiles.
"""
import sys

sys.path.insert(0, "/opt/trn_rl_repo")

import numpy as np
import concourse.bass as bass
import concourse.bass_isa as bass_isa
from concourse import bacc
import concourse.mybir as mybir
import concourse.tile as tile
from concourse.bass_utils import run_bass_kernel_spmd
from concourse.masks import make_identity
from contextlib import ExitStack

P = 128
B, T, E = 2, 2048, 2048
NH = 4            # q heads per core
D = 128           # head dim
DQ = NH * D       # per-core q width
DKV = 128         # per-core kv width
TT = 512          # t-tile (projection output / attention query tile)
NTT = T // TT     # 4
NJT = T // P      # 16 key tiles
NEC = E // P      # 16 contraction chunks
NQ = 8            # x chunks per t-tile in phase 1
EC4 = NEC // NQ   # 2
EPS = 1e-6
ROPE_BASE = 100000.0
MASKV = -10000.0
F32 = mybir.dt.float32
BF = mybir.dt.bfloat16
AF = mybir.ActivationFunctionType

_cache = {}


def _build():
    nc = bacc.Bacc("TRN2", target_bir_lowering=False, debug=False)
    # all inputs host-rearranged to partition-contiguous layouts so every
    # DMA is one descriptor per partition
    xT_d = nc.dram_tensor("xT", [P, NTT, NEC, TT], BF, kind="ExternalInput")
    wq_d = nc.dram_tensor("wq", [P, NEC, DQ], BF, kind="ExternalInput")
    wk_d = nc.dram_tensor("wk", [P, NEC, DKV], BF, kind="ExternalInput")
    wv_d = nc.dram_tensor("wv", [P, NEC, DKV], BF, kind="ExternalInput")
    wo_d = nc.dram_tensor("wo", [P, DQ // P, E], BF, kind="ExternalInput")
    cos_d = nc.dram_tensor("cosf", [P, T], F32, kind="ExternalInput")
    sin_d = nc.dram_tensor("sinf", [P, T], F32, kind="ExternalInput")
    msk_d = nc.dram_tensor("msk", [P, P], BF, kind="ExternalInput")
    ones_d = nc.dram_tensor("ones", [P, 1], BF, kind="ExternalInput")
    out_d = nc.dram_tensor("outT", [E, T], BF, kind="ExternalOutput")

    H = D // 2  # rope half

    with ExitStack() as ctx:
        tc = ctx.enter_context(tile.TileContext(nc))
        persist = ctx.enter_context(tc.tile_pool(name="persist", bufs=1))

        # ---- persistent tiles (both phases) ----
        # strict-lower -10000 additive mask for the diagonal window
        msk_sb = persist.tile([P, P], BF, tag="msk")
        ones_sb = persist.tile([P, 1], BF, tag="ones")
        # per-t-tile q/k/v so attention deps don't serialize on later tiles
        qT_t = [persist.tile([P, NH, TT], BF, tag=f"qT{t}", name=f"qT{t}")
                for t in range(NTT)]
        kT_t = [persist.tile([P, TT], BF, tag=f"kT{t}", name=f"kT{t}")
                for t in range(NTT)]
        v_t = [persist.tile([P, 4, D], BF, tag=f"v{t}", name=f"v{t}")
               for t in range(NTT)]
        epsq_sb = persist.tile([P, 1], F32, tag="epsq")
        nc.vector.memset(epsq_sb, EPS)
        identb_sb = persist.tile([P, P], BF, tag="identb")
        make_identity(nc, identb_sb)
        identf_sb = persist.tile([P, P], F32, tag="identf")
        make_identity(nc, identf_sb)
        # bridge tiles: t-tile 3's projections + rope tables survive the
        # phase-1 pool scope; its rope pipeline runs inside phase 2
        pqs3 = [persist.tile([P, TT], F32, tag=f"pqs3_{i}", name=f"pqs3_{i}")
                for i in range(5)]
        ct3 = persist.tile([P, TT], F32, tag="ct3")
        st3 = persist.tile([P, TT], F32, tag="st3")
        vt3 = persist.tile([P, TT], BF, tag="vt3")

        def rope_rows(pqs, qsq, dst, ctt, stt, scr, rows):
            """rms factor + rope from drained projections (pqs) -> dst.
            partition_all_reduce leaves the sum in every partition, so the
            rms scale is computed full-tile (ACT sqrt + DVE recip) with no
            row extraction / broadcast stage."""
            ssb = scr.tile([P, TT], F32, tag="ssb")
            nc.gpsimd.partition_all_reduce(ssb, qsq, channels=P,
                                           reduce_op=bass_isa.ReduceOp.add)
            srt = scr.tile([P, TT], F32, tag="srt")
            nc.scalar.activation(srt, ssb, AF.Sqrt,
                                 bias=epsq_sb, scale=1.0 / D)
            rbc = scr.tile([P, TT], F32, tag="rbc")
            nc.vector.reciprocal(rbc, srt)
            rp = scr.tile([P, TT], F32, tag="rp")
            rt = scr.tile([P, TT], F32, tag="rt")
            nc.vector.tensor_mul(rp, pqs, ctt)
            nc.vector.tensor_mul(rt[0:H], pqs[H:D], stt[H:D])
            nc.vector.tensor_mul(rt[H:D], pqs[0:H], stt[0:H])
            nc.vector.tensor_add(rp, rp, rt)
            nc.vector.tensor_mul(dst, rp, rbc)  # f32 -> bf16 cast

        # ================= phase 1: projections (+ rope for tt 0-2) =========
        with tc.tile_pool(name="wpool", bufs=1) as wpool, \
             tc.tile_pool(name="xpool", bufs=6) as xpool, \
             tc.tile_pool(name="csp", bufs=2) as csp, \
             tc.tile_pool(name="work1", bufs=4) as work1, \
             tc.tile_pool(name="rows1", bufs=3) as rows1, \
             tc.tile_pool(name="ps_q", bufs=7, space="PSUM") as ps_q, \
             tc.tile_pool(name="ps_vt", bufs=1, space="PSUM") as ps_vt:

            wq_sb = wpool.tile([P, NEC, DQ], BF, tag="wq")
            wk_sb = wpool.tile([P, NEC, DKV], BF, tag="wk")
            wv_sb = wpool.tile([P, NEC, DKV], BF, tag="wv")
            wrs = [(w_sb, w_d.ap())
                   for w_sb, w_d in ((wq_sb, wq_d), (wk_sb, wk_d), (wv_sb, wv_d))]

            def wgroup(lo, n):  # stage weight chunks lo..lo+n
                for w_sb, wr in wrs:
                    nc.sync.dma_start(w_sb[:, lo : lo + n, :],
                                      wr[:, lo : lo + n, :])

            for tt in range(NTT):
                ts_ = slice(tt * TT, (tt + 1) * TT)
                if tt == 3:
                    ctt, stt = ct3, st3
                else:
                    ctt = csp.tile([P, TT], F32, tag="ctt")
                    stt = csp.tile([P, TT], F32, tag="stt")
                if tt != 0:
                    nc.sync.dma_start(ctt, cos_d.ap()[:, ts_])
                    nc.sync.dma_start(stt, sin_d.ap()[:, ts_])
                else:
                    # first-need DMAs lead the queue: wq chunks 0-1 + first
                    # x tile gate the first matmul; everything else follows
                    nc.sync.dma_start(wq_sb[:, 0:2, :], wrs[0][1][:, 0:2, :])

                # 6 co-accumulating chains: 4 q heads, k, v
                chains = [(wq_sb, h * D, (h + 1) * D) for h in range(NH)]
                chains.append((wk_sb, 0, DKV))
                chains.append((wv_sb, 0, DKV))
                pts = [ps_q.tile([P, TT], F32, tag="q", name=f"pq{i}")
                       for i in range(6)]
                for c4 in range(NQ):
                    xq = xpool.tile([P, EC4, TT], BF, tag="xh")
                    nc.sync.dma_start(
                        xq, xT_d.ap()[:, tt, c4 * EC4 : (c4 + 1) * EC4, :])
                    # stream remaining weight groups behind the early x tiles
                    if tt == 0:
                        if c4 == 0:
                            nc.sync.dma_start(msk_sb, msk_d.ap())
                            nc.sync.dma_start(ones_sb, ones_d.ap())
                            nc.sync.dma_start(wk_sb[:, 0:4, :], wrs[1][1][:, 0:4, :])
                            nc.sync.dma_start(wv_sb[:, 0:4, :], wrs[2][1][:, 0:4, :])
                            nc.sync.dma_start(wq_sb[:, 2:4, :], wrs[0][1][:, 2:4, :])
                            nc.sync.dma_start(ctt, cos_d.ap()[:, ts_])
                            nc.sync.dma_start(stt, sin_d.ap()[:, ts_])
                        elif c4 in (1, 3, 5):
                            wgroup(4 * (c4 + 1) // 2, 4)
                    for ci, (w_sb, lo, hi) in enumerate(chains):
                        # tt0/c4=0 runs q-chains only (wk/wv DMAs trail the
                        # first x tile); k/v catch up at c4==1 while xq(0)
                        # is still resident
                        if tt == 0 and ci >= 4:
                            if c4 == 0:
                                continue
                            ecs = range(4) if c4 == 1 else                                 range(c4 * EC4, (c4 + 1) * EC4)
                        else:
                            ecs = range(c4 * EC4, (c4 + 1) * EC4)
                        for ec in ecs:
                            xqt = xq if ec // EC4 == c4 else xq_prev
                            nc.tensor.matmul(pts[ci], w_sb[:, ec, lo:hi],
                                             xqt[:, ec % EC4, :],
                                             start=(ec == 0),
                                             stop=(ec == NEC - 1))
                    xq_prev = xq

                # v: drain its psum first (single ACT copy, frees the bank)
                pv = pts[5]
                vtmp = vt3 if tt == 3 else work1.tile([P, TT], BF, tag="vtmp")
                nc.scalar.copy(vtmp, pv)

                if tt == 3:
                    # boundary: only drain the psums (split ACT/DVE so the
                    # ACT FIFO stays short); rope runs inside phase 2
                    for i, ci in enumerate((3, 4, 0, 1, 2)):
                        if i < 2:
                            nc.scalar.copy(pqs3[ci], pts[ci])
                        else:
                            nc.vector.tensor_copy(pqs3[ci], pts[ci])
                    for j4 in range(TT // P):
                        pvt = ps_vt.tile([P, P], BF, tag="vt")
                        nc.tensor.transpose(pvt, vt3[:, j4 * P : (j4 + 1) * P],
                                            identb_sb)
                        nc.vector.tensor_copy(v_t[3][:, j4, :], pvt)
                    continue

                # q heads and k share the same rms+rope pipeline
                dsts = [qT_t[tt][:, h, :] for h in range(NH)] + [kT_t[tt]]
                for ci in range(5):
                    pq = pts[ci]
                    # ACT drains psum (copy + square) so the psum slot frees
                    # fast and all DVE rope ops run SBUF-only (2x mode)
                    pqs = work1.tile([P, TT], F32, tag="pqs")
                    nc.scalar.copy(pqs, pq)
                    qsq = work1.tile([P, TT], F32, tag="qsq")
                    nc.scalar.square(qsq, pq)
                    rope_rows(pqs, qsq, dsts[ci], ctt, stt, work1, rows1)

                # v: PE-transpose into [t, d] chunks (bf16: 1 cycle/row)
                for j4 in range(TT // P):
                    pvt = ps_vt.tile([P, P], BF, tag="vt")
                    nc.tensor.transpose(pvt, vtmp[:, j4 * P : (j4 + 1) * P],
                                        identb_sb)
                    nc.vector.tensor_copy(v_t[tt][:, j4, :], pvt)

        # ================= phase 2+3: attention + o_proj =================
        with tc.tile_pool(name="wop", bufs=1) as wop, \
             tc.tile_pool(name="ytp", bufs=3) as ytp, \
             tc.tile_pool(name="expp", bufs=8) as expp, \
             tc.tile_pool(name="ostg", bufs=6) as ostg, \
             tc.tile_pool(name="work2", bufs=3) as work2, \
             tc.tile_pool(name="rows2", bufs=3) as rows2, \
             tc.tile_pool(name="ps_s", bufs=3, space="PSUM") as ps_s, \
             tc.tile_pool(name="ps_y", bufs=2, space="PSUM") as ps_y, \
             tc.tile_pool(name="ps_d", bufs=1, space="PSUM") as ps_d, \
             tc.tile_pool(name="ps_o", bufs=2, space="PSUM") as ps_o:

            wo_sb = wop.tile([P, DQ // P, E], BF, tag="wo")
            nc.sync.dma_start(wo_sb, wo_d.ap())

            def finalize(h, yt, py, pd4):
                # 1/d back to a broadcast row: per-column PE transposes into
                # partition-0 rows (PSUM reads must be 32-aligned), then one
                # ACT copy to SBUF, gpsimd broadcast, and the normalization
                rq = work2.tile([P, 4], F32, tag="rq")
                nc.vector.reciprocal(rq, pd4)
                prt = ps_s.tile([P, TT], F32, tag="s", name="prt")
                for c in range(4):
                    # single start per bank: start=True lazily re-zeroes the
                    # whole 2KB zero-region, clobbering sibling columns
                    nc.tensor.matmul(prt[0:1, c * P : (c + 1) * P],
                                     rq[:, c : c + 1], identf_sb,
                                     is_transpose=True, start=(c == 0),
                                     stop=(c == 3), skip_group_check=True)
                rdrow = rows2.tile([1, TT], F32, tag="rdrow")
                nc.vector.tensor_copy(rdrow, prt[0:1, 0:TT])
                rdb = work2.tile([P, TT], F32, tag="rdb")
                nc.gpsimd.partition_broadcast(rdb, rdrow)
                nc.vector.tensor_mul(yt[:, h, :], py, rdb)  # f32 -> bf16

            def oproj_tiles(it, yt, es):
                its = slice(it * TT, (it + 1) * TT)
                for e in es:
                    po = ps_o.tile([P, TT], F32, tag="o")
                    for c in range(DQ // P):
                        nc.tensor.matmul(po, wo_sb[:, c, e * P : (e + 1) * P],
                                         yt[:, c, :], start=(c == 0),
                                         stop=(c == DQ // P - 1))
                    og = ostg.tile([P, TT], BF, tag="og")
                    # DVE drains: ACT is the scarce engine in phase 2
                    nc.vector.tensor_copy(og, po)
                    nc.sync.dma_start(out_d.ap()[e * P : (e + 1) * P, its], og)

            def emit_head(it, h, yt):
                py = ps_y.tile([P, TT], F32, tag="y")
                pd4 = ps_d.tile([P, 4], F32, tag="d")
                # diagonal tiles first (ascending o), trimmed to the valid
                # query columns, then the full non-diag tiles
                seq = [(4 * it + o, TT - P * o, P * o) for o in range(4)]
                seq += [(jt, TT, 0) for jt in range(4 * it)]
                n = len(seq)
                for si, (jt, w, lo) in enumerate(seq):
                    first, last = si == 0, si == n - 1
                    diag = jt >= 4 * it
                    o_ = jt - 4 * it if diag else 0
                    pss_ = ps_s.tile([P, TT], F32, tag="s")
                    nc.tensor.matmul(pss_[:, :w],
                                     kT_t[jt // 4][:, (jt % 4) * P : (jt % 4 + 1) * P],
                                     qT_t[it][:, h, lo:TT],
                                     start=True, stop=not diag,
                                     skip_group_check=True)
                    if diag:
                        # accumulate the causal mask on the PE: strict
                        # lower -10000 on the leading 128-col window
                        nc.tensor.matmul(pss_[:, 0:P], identb_sb,
                                         msk_sb[:, 0:P],
                                         start=False, stop=True,
                                         skip_group_check=True)
                    ex = expp.tile([P, TT], BF, tag="ex")
                    nc.scalar.activation(ex[:, :w], pss_[:, :w], AF.Exp,
                                         scale=float(1.0 / np.sqrt(D)))
                    nc.tensor.matmul(py[:, lo:TT], v_t[jt // 4][:, jt % 4, :],
                                     ex[:, :w], start=first, stop=last,
                                     skip_group_check=True)
                    # denominator: free-size-1 column matmuls per q chunk
                    for c in range(4):
                        if c < o_:
                            continue
                        loc = P * (c - o_) if diag else P * c
                        # single start per bank (see finalize note)
                        nc.tensor.matmul(pd4[:, c : c + 1],
                                         ex[:, loc : loc + P], ones_sb,
                                         start=first and c == 0,
                                         stop=last,
                                         skip_group_check=True)
                return (h, yt, py, pd4)

            rbc3 = []

            def late_rope_tt3_stage1():
                # rms factors for t-tile 3: Pool/ACT/DVE work that drains
                # during the block-A attention stretch
                for ci in range(5):
                    qsq = work2.tile([P, TT], F32, tag="qsq3")
                    nc.vector.tensor_mul(qsq, pqs3[ci], pqs3[ci])
                    ssb = work2.tile([P, TT], F32, tag="ssb")
                    nc.gpsimd.partition_all_reduce(ssb, qsq, channels=P,
                                                   reduce_op=bass_isa.ReduceOp.add)
                    srt = work2.tile([P, TT], F32, tag="srt")
                    nc.scalar.activation(srt, ssb, AF.Sqrt,
                                         bias=epsq_sb, scale=1.0 / D)
                    rbc = persist.tile([P, TT], F32, tag=f"rbc3_{ci}",
                                       name=f"rbc3_{ci}")
                    nc.vector.reciprocal(rbc, srt)
                    rbc3.append(rbc)

            def late_rope_tt3():
                dsts = [qT_t[3][:, h, :] for h in range(NH)] + [kT_t[3]]
                for ci in range(5):
                    rp = work2.tile([P, TT], F32, tag="rp")
                    rt = work2.tile([P, TT], F32, tag="rt")
                    nc.vector.tensor_mul(rp, pqs3[ci], ct3)
                    nc.vector.tensor_mul(rt[0:H], pqs3[ci][H:D], st3[H:D])
                    nc.vector.tensor_mul(rt[H:D], pqs3[ci][0:H], st3[0:H])
                    nc.vector.tensor_add(rp, rp, rt)
                    nc.vector.tensor_mul(dsts[ci], rp, rbc3[ci])
            # driver: attention heads of tiles 0/1 pair-interleaved (evens
            # the ACT-heavy early profile), o_proj(0)+(1) e-tiles fill
            # attention(2), o_proj(2) fills attention(3), o_proj(3) tails.
            # Each head's finalize defers one slot so its latency hides.
            ytd = {}
            pend = [None]
            fillq = []

            def head_slot(it, h, fill_n):
                if it not in ytd:
                    ytd[it] = ytp.tile([P, NH, TT], BF, tag="yt",
                                       name=f"yt{it}")
                rec = emit_head(it, h, ytd[it])
                if pend[0] is not None:
                    finalize(*pend[0])
                for _ in range(min(fill_n, len(fillq))):
                    it2, e = fillq.pop(0)
                    oproj_tiles(it2, ytd[it2], [e])
                pend[0] = rec

            for h in range(NH):
                head_slot(1, h, 0)
                head_slot(0, h, 0)
                if h == 1:
                    late_rope_tt3_stage1()
            fillq += [(0, e) for e in range(NEC)]
            fillq += [(1, e) for e in range(NEC)]
            head_slot(2, 0, 8)
            head_slot(2, 1, 8)
            late_rope_tt3()
            head_slot(2, 2, 8)
            head_slot(2, 3, 8)
            fillq += [(2, e) for e in range(NEC)]
            head_slot(3, 0, 4)
            head_slot(3, 1, 4)
            head_slot(3, 2, 4)
            head_slot(3, 3, 4)
            finalize(*pend[0])
            oproj_tiles(3, ytd[3], range(NEC))

    nc.compile()
    return nc


def _tables():
    half = D // 2
    inv_freq = 1.0 / (ROPE_BASE ** (np.arange(half, dtype=np.float64) / half))
    freqs = np.arange(T, dtype=np.float64)[:, None] * inv_freq[None, :]  # [T, half]
    cosT = np.cos(freqs).T.astype(np.float32)  # [half, T]
    sinT = np.sin(freqs).T.astype(np.float32)
    cos_full = np.concatenate([cosT, cosT], axis=0)          # [P, T]
    # rows 0:64 hold -sinT (multiplies x1 into out[64:128]), rows 64:128
    # hold +sinT (multiplies x2 into out[0:64]) so DVE input base
    # partitions match the swapped-half reads.
    sin_signed = np.concatenate([-sinT, sinT], axis=0)       # [P, T]
    # additive strict-lower MASKV mask for the diagonal window
    msk = np.where(np.arange(P)[None, :] < np.arange(P)[:, None],
                   MASKV, 0.0).astype(np.float32)
    return cos_full, sin_signed, msk


def kernel(x, wq, wk, wv, wo):
    import ml_dtypes
    bf = ml_dtypes.bfloat16
    if "nc" not in _cache:
        _cache["nc"] = _build()
    nc = _cache["nc"]
    cos_full, sin_signed, msk = _tables()
    def colshard(w, g, width):  # [E, M] -> [P, NEC, width] chunk layout
        ws = w[:, g * width : (g + 1) * width].astype(bf)
        return np.ascontiguousarray(ws.reshape(NEC, P, width).transpose(1, 0, 2))

    xTs = []
    for b in range(B):
        xT = np.asarray(x)[b].T.astype(bf)  # [E, T]
        xTs.append(np.ascontiguousarray(
            xT.reshape(NEC, P, NTT, TT).transpose(1, 2, 0, 3)))
    wq, wk, wv, wo = (np.asarray(a, dtype=np.float32) for a in (wq, wk, wv, wo))
    in_maps = []
    for c in range(8):
        b, g = divmod(c, 4)
        wos = wo[g * DQ : (g + 1) * DQ, :].astype(bf)
        in_maps.append({
            "xT": xTs[b],
            "wq": colshard(wq, g, DQ),
            "wk": colshard(wk, g, DKV),
            "wv": colshard(wv, g, DKV),
            "wo": np.ascontiguousarray(
                wos.reshape(DQ // P, P, E).transpose(1, 0, 2)),
            "cosf": cos_full,
            "sinf": sin_signed,
            "msk": msk.astype(bf),
            "ones": np.ones((P, 1), dtype=bf),
        })
    res = run_bass_kernel_spmd(nc, in_maps, core_ids=list(range(8)))
    out = np.zeros((B, T, E), dtype=np.float32)
    for c in range(8):
        b = c // 4
        out[b] += res.results[c]["outT"].astype(np.float32).T
    return out
